# revision 35
# baseline (speedup 1.0000x reference)
"""Trainium2 Bass kernel for sliding-window GQA attention (qk-norm + RoPE).

Problem: B=2, S=2048, D=1024, 16 heads / 4 kv heads, head_dim 64,
causal sliding window 512, fp32 I/O.

Sharding: 8 cores = batch(2) x sequence(4). Each core computes 512 query
tokens against a 1024-token context window (512-token halo; chunk 0 is
zero-padded on the left). Fully data-parallel SPMD - no collectives.

Per-core dataflow, built around a transposed score layout so softmax
needs no transposes:
  xT (host-pre-transposed to [quarter, partition, dim-slab, token] so
  every DMA slab is a contiguous 4KB-per-partition burst) -> fused k|v
  projection (wk and wv concatenated on the host: ONE matmul per dim
  slab, a single PSUM accumulation group per 128-token context tile -
  note start=True clears PSUM has_written per BANK, so two interleaved
  accumulation groups must never share a bank) -> per-head RMSNorm +
  RoPE -> PE-transpose q/k to [dim, token].
  RMSNorm/RoPE: RoPE commutes with the per-token 1/rms scale, so the
  norm branch (square/reduce/1/rms) and the rope branch (4 muls+add/sub
  on RAW values, tables host-precomputed with qn/kn folded in) run in
  parallel across engines and join in two scale multiplies. 1/rms is
  computed as exp(-0.5*ln(mean+eps)) on Act: Ln and Exp share ONE
  activation table set with the attention exps, where a Sqrt would force
  two ~2.7us ACT_TABLE_LOADs around every attention exp burst (the
  single biggest HW-vs-cost-model divergence found).
  Attention is group-batched: one score matmul covers all 4 q-heads of a
  kv group (moving operand [64 dims, 4 heads, 128 q] = N=512), sliding-
  window triangle masks are added on the PE as a second accumulate matmul
  (mT.T @ [I I I I]), exp runs on Act per context tile. attn@v uses v
  tiles augmented with 64 REPLICATED validity columns (DMA'd from a
  host-precomputed buffer), so the softmax denominator materializes
  already broadcast across 64 PSUM partitions in the same matmul as the
  numerator: vA = [v | den*64] for even groups, vB = [den*64 | v] for
  odd. Normalization is then a DVE reciprocal straight out of PSUM
  (partition-shifted write - legal when the input is PSUM) followed by a
  single fused multiply that both normalizes and evacuates the numerator
  PSUM->SBUF. No separate denominator copy, no PE broadcast matmul, no
  standalone oT copy.
  Scheduling: kv-projection matmuls stream back-to-back on the PE (the
  pp ring double-buffers whole kv tiles) while each tile's norm/rope
  chain runs concurrently on Act/DVE/Pool; all PE transposes are emitted
  at the end of the preamble, after their chains have completed, so the
  in-order PE queue never stalls on them. Context tiles 5..7 norm and
  transpose lazily inside the attention loop (their kT is first read at
  qb=1..3), leaving the first score matmul gated only by q0's own chain.
  In the loop, attn@v of pair i issues after the scores of pair i+1, and
  the next q projection plus the previous block's out-projection fill PE
  gaps so the Act exp queue never starves. Transpose results are staged
  in batched PSUM slabs and evacuated with one DVE copy per slab. A few
  tiny warm-up transposes tied to early DMA arrivals keep the PE_HAM
  activity monitor from re-throttling the clock during the DMA head.
  The last query block's out-projection is split into jt-halves so its
  first half runs as soon as groups 0/1 are normalized, and the y DMAs
  stream per half, shortening the serial tail.

Head-slot permutation: q heads are permuted on the host so every head's
64 q-rows sit at the same SBUF partition offset (0 or 64) as its kv
group's k-rows - matmul requires lhsT/rhs base partitions to match.
wo rows are permuted to match. The within-head dims of q/k are permuted
evens-first so RoPE becomes two contiguous 32-wide halves (scores are
invariant to a shared q/k dim permutation).

build_program(loop_n=K) wraps the body in a hardware For_i loop (used by
test.py to measure true per-execution device time via a two-point slope,
cancelling this environment's ~70 ms per-dispatch tunnel latency).
"""

import sys

sys.path.insert(0, "/opt/trn_rl_repo")

from contextlib import ExitStack

import numpy as np
import ml_dtypes

import bass_rust
import concourse.bass as bass
import concourse.tile as tile
from concourse import mybir

# ---------------- problem constants ----------------
B, S, D = 2, 2048, 1024
H, KV, HD = 16, 4, 64
WINDOW = 512
EPS = 1e-5
NCORES = 8
TQ = 512          # query tokens per core
TC = 1024         # context tokens per core (incl. 512 halo)
NQT = TQ // 128   # 4 query tiles
NCT = TC // 128   # 8 context tiles
P = 128

F32 = mybir.dt.float32
BF16 = mybir.dt.bfloat16
F32R = mybir.dt.float32r
ALU = mybir.AluOpType
ACTF = mybir.ActivationFunctionType

# q-head -> slot permutation with parity matching:
# slot p must satisfy p%2 == (head//4)%2 so that the q rows (at partition
# offset (p%2)*64) align with the kv group's k rows.
HEAD_OF_SLOT = [0, 4, 1, 5, 2, 6, 3, 7, 8, 12, 9, 13, 10, 14, 11, 15]


def split_multiwaits(nc):
    """This environment's walrus build rejects any instruction with more
    than one sync-wait condition. Split extras into preceding single-wait
    NoOps on the same engine (identical blocking semantics)."""
    n_split = 0
    for f in nc.m.functions:
        for blk in f.blocks:
            out = []
            changed = False
            for inst in blk.instructions:
                try:
                    si = inst.sync_info
                    waits = list(si.on_wait)
                except Exception:
                    out.append(inst)
                    continue
                if len(waits) > 1:
                    changed = True
                    for j, w in enumerate(waits[:-1]):
                        nop = mybir.InstNoOp(
                            name=f"{inst.name}-wsplit{j}", ins=[], outs=[])
                        nop.engine = inst.engine
                        nop.sync_info = bass_rust.SyncInfo(
                            on_wait=[w], on_update=[])
                        nc.register_instruction(nop, overwrite=True)
                        out.append(nop)
                        n_split += 1
                    inst.sync_info = bass_rust.SyncInfo(
                        on_wait=[waits[-1]], on_update=list(si.on_update))
                out.append(inst)
            if changed:
                blk.instructions = out
    return n_split


# ---------------- program builder ----------------

def emit(nc, tc, ctx, stage=3):
    """stage (timing-only knob): 0 = input DMAs only, 1 = + projections/
    norm/rope/transposes, 2 = + attention, 3 = full (+ out-projection).
    stage<3 leaves y_sb unwritten - outputs are garbage, timing variants
    only."""
    cp = ctx.enter_context(tc.tile_pool(name="const", bufs=1))
    pp = ctx.enter_context(tc.tile_pool(name="pp", bufs=2, space="PSUM"))
    ntp = ctx.enter_context(tc.tile_pool(name="ntp", bufs=2, space="PSUM"))
    stp = ctx.enter_context(tc.tile_pool(name="stp", bufs=2, space="PSUM"))
    scr = ctx.enter_context(tc.tile_pool(name="scr", bufs=4))
    epool = ctx.enter_context(tc.tile_pool(name="epool", bufs=5))
    rpool = ctx.enter_context(tc.tile_pool(name="rpool", bufs=3))

    # DRAM params
    xt_d = nc.declare_dram_parameter("xt", [4, P, 8, 256], BF16, isOutput=False)
    wq_d = nc.declare_dram_parameter("wq", [D, H * HD], BF16, isOutput=False)
    wkv_d = nc.declare_dram_parameter("wkv", [D, 512], BF16, isOutput=False)
    wo_d = nc.declare_dram_parameter("wo", [H * HD, D], BF16, isOutput=False)
    rope_d = nc.declare_dram_parameter("rope", [P, 2 * (NQT + NCT), HD], BF16,
                                       isOutput=False)
    vden_d = nc.declare_dram_parameter("vden", [P, NCT, 64], BF16,
                                       isOutput=False)
    y_d = nc.declare_dram_parameter("y", [TQ, D], F32, isOutput=True)

    # persistent SBUF
    xt = cp.tile([P, 8, TC], BF16, tag="xt")
    wq = cp.tile([P, 8, 1024], BF16, tag="wq")
    wkv = cp.tile([P, 8, 512], BF16, tag="wkv")
    wo = cp.tile([P, 8, 1024], BF16, tag="wo")
    ropet = cp.tile([P, 2 * (NQT + NCT), HD], BF16, tag="ropet")
    cosq = ropet[:, 0:NQT]
    sinq = ropet[:, NQT:2 * NQT]
    cosk = ropet[:, 2 * NQT:2 * NQT + NCT]
    sink = ropet[:, 2 * NQT + NCT:]
    qT = cp.tile([P, 8, TQ], BF16, tag="qT")       # [j, jt, a]
    kT = cp.tile([P, 2, TC], BF16, tag="kT")       # [j, jt2, p]
    vA = cp.tile([P, NCT, KV, 128], BF16, tag="vA")  # v | den-cols @64:
    vB = cp.tile([P, NCT, KV, 128], BF16, tag="vB")  # den-cols | v @64:
    q_raw = cp.tile([P, NQT, 1024], BF16, tag="qraw")
    qrot = cp.tile([P, NQT, 1024], BF16, tag="qrot")
    k_raw = cp.tile([P, NCT, 256], BF16, tag="kraw")
    krot = cp.tile([P, NCT, 256], BF16, tag="krot")
    oT = cp.tile([P, 8, TQ], BF16, tag="oT")
    y_sb = cp.tile([P, NQT, 1024], F32, tag="ysb")
    ident = cp.tile([P, P], BF16, tag="ident")
    mT0 = cp.tile([P, P], BF16, tag="mT0")
    mT4 = cp.tile([P, P], BF16, tag="mT4")
    eps_c = cp.tile([P, 1], F32, tag="epsc")
    ssq_q = cp.tile([P, NQT, H], F32, tag="ssqq")
    ssq_k = cp.tile([P, NCT, KV], F32, tag="ssqk")

    # ---- input DMAs (ordered so kv-proj can start ASAP; wq halves land
    # before the first q projection's dt ranges need them) ----
    wkv_r = wkv_d.rearrange("(a p) n -> p a n", p=P)
    wq_r = wq_d.rearrange("(a p) n -> p a n", p=P)

    def xt_dma(tq_):
        nc.sync.dma_start(xt[:, :, tq_ * 256:(tq_ + 1) * 256], xt_d[tq_])
    # first kv tile's operands land in ~4 us: wkv and the first x quarter
    # are split into dim-halves (keeping 2KB+ contiguous DRAM lines) so
    # mm(0) starts as early as possible
    nc.sync.dma_start(wkv[:, 0:4], wkv_r[:, 0:4])
    nc.sync.dma_start(xt[:, 0:4, 0:256], xt_d[0][:, 0:4])
    nc.sync.dma_start(wkv[:, 4:8], wkv_r[:, 4:8])
    nc.sync.dma_start(xt[:, 4:8, 0:256], xt_d[0][:, 4:8])
    xt_dma(1)
    nc.sync.dma_start(ropet[:], rope_d[:])
    xt_dma(2)
    nc.sync.dma_start(wq[:, 0:4], wq_r[:, 0:4])
    nc.sync.dma_start(wq[:, 4:8], wq_r[:, 4:8])
    xt_dma(3)
    for g_ in range(KV):
        nc.sync.dma_start(vA[:, :, g_, 64:128], vden_d[:])
        nc.sync.dma_start(vB[:, :, g_, 0:64], vden_d[:])

    # ---- on-chip constants ----
    # identity for PE transposes
    nc.gpsimd.memset(ident[:], 0.0)
    nc.gpsimd.memset(eps_c[:], EPS)
    nc.gpsimd.affine_select(
        out=ident[:], in_=ident[:], compare_op=ALU.not_equal, fill=1.0,
        base=0, pattern=[[-1, P]], channel_multiplier=1)
    # additive sliding-window masks, applied on the PE as an extra
    # accumulate matmul (out += mT.T @ [I I I I]) inside the score group.
    # mask tile 0 keeps a < kp: add -30000 where a >= kp
    nc.gpsimd.memset(mT0[:], 0.0)
    nc.gpsimd.affine_select(
        out=mT0[:], in_=mT0[:], compare_op=ALU.is_gt, fill=-30000.0,
        base=0, pattern=[[1, P]], channel_multiplier=-1)
    # mask tile 4 keeps a >= kp: add -30000 where a < kp
    nc.gpsimd.memset(mT4[:], 0.0)
    nc.gpsimd.affine_select(
        out=mT4[:], in_=mT4[:], compare_op=ALU.is_ge, fill=-30000.0,
        base=0, pattern=[[-1, P]], channel_multiplier=1)


    # HAM warm-up: keep the PE activity monitor from re-throttling the
    # clock during the per-iteration DMA head - tiny transposes tied to
    # successive input arrivals (results unused).
    for warm_src in (ident[:, 0:P], wkv[:, 0, 0:P], xt[:, 0, 0:P]):
        wtp = ntp.tile([P, P], BF16, tag="ntp")
        nc.tensor.transpose(wtp[:], warm_src, ident[:])

    inv64 = 1.0 / 64.0

    def rmsnorm_rope(raw, rot, nt, nh, ssq, cosT, sinT, it):
        """raw/rot: [P, nt, nh*64] bf16 slabs; process tile `it`.
        RoPE commutes with the per-token 1/rms scale, so the norm branch
        (square/reduce/sqrt/recip) and the rope branch (4 muls + add/sub
        on RAW values) run in parallel across engines and join in two
        final scale multiplies - about half the serial chain latency of
        norm-then-rope."""
        hv = raw[:, it].rearrange("p (h d) -> p h d", h=nh)
        rv = rot[:, it].rearrange("p (h d) -> p h d", h=nh)
        # norm branch
        s2 = scr.tile([P, 1024], BF16, tag="s2")
        s2v = s2[:, 0:nh * HD].rearrange("p (h d) -> p h d", h=nh)
        nc.vector.tensor_mul(s2v[:], hv[:], hv[:])
        nc.vector.tensor_reduce(
            out=ssq[:, it], in_=s2v[:], axis=mybir.AxisListType.X,
            op=ALU.add)
        # 1/rms = exp(-0.5*ln(mean+eps)): Ln and Exp live in the SAME Act
        # table set (natural_log_exp_and_others) as the attention exps, so
        # the set stays resident - a Sqrt here would force two ~2.7us
        # ACT_TABLE_LOADs around every attention exp burst.
        lg = scr.tile([P, nh], F32, tag="sq")
        nc.scalar.activation(lg[:], ssq[:, it], ACTF.Ln,
                             bias=eps_c[:], scale=inv64)
        rsb = scr.tile([P, nh], BF16, tag="rsb")
        nc.scalar.activation(rsb[:], lg[:], ACTF.Exp, bias=0.0, scale=-0.5)
        # rope branch on raw values
        yA = hv[:, :, 0:32]
        yB = hv[:, :, 32:64]
        cA = cosT[:, it:it + 1, 0:32].broadcast_to([P, nh, 32])
        cB = cosT[:, it:it + 1, 32:64].broadcast_to([P, nh, 32])
        sA = sinT[:, it:it + 1, 0:32].broadcast_to([P, nh, 32])
        sB = sinT[:, it:it + 1, 32:64].broadcast_to([P, nh, 32])
        w = nh * 32

        def half(tag):
            r = scr.tile([P, 512], BF16, tag=tag)
            return r[:, 0:w].rearrange("p (h d) -> p h d", h=nh)
        r1v, r2v, r3v, r4v = (half(t) for t in ("r1", "r2", "r3", "r4"))
        rlov = half("rlo")
        rhiv = half("rhi")
        nc.vector.tensor_mul(r1v[:], yA, cA)
        nc.vector.tensor_mul(r2v[:], yB, sA)
        nc.vector.tensor_tensor(out=rlov[:], in0=r1v[:], in1=r2v[:],
                                op=ALU.subtract)
        nc.gpsimd.tensor_mul(r3v[:], yB, cB)
        nc.gpsimd.tensor_mul(r4v[:], yA, sB)
        nc.gpsimd.tensor_tensor(out=rhiv[:], in0=r3v[:], in1=r4v[:],
                                op=ALU.add)
        # join: scale both halves by 1/rms
        rsbb = rsb[:].unsqueeze(2).broadcast_to([P, nh, 32])
        nc.vector.tensor_tensor(out=rv[:, :, 0:32], in0=rlov[:], in1=rsbb,
                                op=ALU.mult)
        nc.gpsimd.tensor_tensor(out=rv[:, :, 32:64], in0=rhiv[:], in1=rsbb,
                                op=ALU.mult)

    # ---- k/v projection unit (one context tile): k and v share one PSUM
    # tile (two element-disjoint accumulation groups) so pp double-buffers
    # whole context tiles ----
    def emit_kv_mm(ct):
        # k and v are projected by ONE matmul per dt against the fused
        # wk|wv weight - a single PSUM accumulation group in one bank
        # (start=True clears has_written per BANK, so two interleaved
        # groups in one bank corrupt each other), and the pp ring then
        # double-buffers whole kv tiles.
        kvps = pp.tile([P, 512], F32, tag="pp")
        kps = kvps[:, 0:256]
        vps = kvps[:, 256:512]
        for dt in range(8):
            lhs = xt[:, dt, ct * P:(ct + 1) * P]
            nc.tensor.matmul(kvps[:], lhs, wkv[:, dt],
                             start=(dt == 0), stop=(dt == 7))
        nc.scalar.copy(k_raw[:, ct], kps[:])
        # v -> vA (cols 0:64 per group) and vB (cols 64:128)
        nc.scalar.copy(
            vA[:, ct, :, 0:64],
            vps[:].rearrange("p (g d) -> p g d", g=KV))
        nc.vector.tensor_copy(
            vB[:, ct, :, 64:128],
            vps[:].rearrange("p (g d) -> p g d", g=KV))

    def emit_kv_norm(ct):
        rmsnorm_rope(k_raw, krot, NCT, KV, ssq_k, cosk, sink, ct)

    def emit_kv_transp(ct):
        # transpose krot tile -> batched PSUM slab -> one DVE copy
        tp = ntp.tile([P, 2, P], BF16, tag="ntp")
        for j2 in range(2):
            nc.tensor.transpose(tp[:, j2], krot[:, ct, j2 * P:(j2 + 1) * P],
                                ident[:])
        nc.vector.tensor_copy(kT[:, :, ct * P:(ct + 1) * P], tp[:])

    # ---- q projection helper, split into schedulable chunks ----
    def qproj_mm(at, qps0, qps1, dts):
        for dt in dts:
            lhs = xt[:, dt, TQ + at * P:TQ + (at + 1) * P]
            nc.tensor.matmul(qps0[:], lhs, wq[:, dt, 0:512],
                             start=(dt == 0), stop=(dt == 7))
            nc.tensor.matmul(qps1[:], lhs, wq[:, dt, 512:1024],
                             start=(dt == 0), stop=(dt == 7))

    def qproj_tail(at, qps0, qps1):
        nc.scalar.copy(q_raw[:, at, 0:512], qps0[:])
        nc.scalar.copy(q_raw[:, at, 512:1024], qps1[:])
        rmsnorm_rope(q_raw, qrot, NQT, H, ssq_q, cosq, sinq, at)

    def qproj_transpose(at, half):
        # 4 transposes -> one PSUM slab -> one DVE copy
        tp = ntp.tile([P, 4, P], BF16, tag="ntp")
        for j in range(4):
            jt = half * 4 + j
            nc.tensor.transpose(tp[:, j], qrot[:, at, jt * P:(jt + 1) * P],
                                ident[:])
        nc.vector.tensor_copy(
            qT[:, half * 4:half * 4 + 4, at * P:(at + 1) * P], tp[:])

    def emit_qproj(at):
        qps0 = pp.tile([P, 512], F32, tag="pp")
        qps1 = pp.tile([P, 512], F32, tag="pp")
        qproj_mm(at, qps0, qps1, range(8))
        qproj_tail(at, qps0, qps1)
        qproj_transpose(at, 0)
        qproj_transpose(at, 1)

    # ---- attention helpers ----
    # Group-batched: one matmul covers all 4 q-heads of a kv group
    # (moving operand [64, 4 heads, 128 q] = N=512). Sliding-window masks
    # are added on the PE inside the score accumulation group.
    e_const = None
    if stage in (5, 6):
        e_const = epool.tile([P, 5, 4, P], BF16, tag='e')
        nc.gpsimd.memset(e_const[:], 0.001)

    def pair_params(g):
        par = g % 2
        return (par * 64,                 # partition offset of the 64 q-dims
                0 if g < 2 else 4)        # jlo: q slots 2*(jlo+i)+par

    def emit_scores(qb, g):
        off, jlo = pair_params(g)
        e = epool.tile([P, 5, 4, P], BF16, tag="e")
        identb4 = ident[:].unsqueeze(1).broadcast_to([P, 4, P])
        for c0, c1 in ((0, 2), (2, 4), (4, 5)):
            n = c1 - c0
            sT = stp.tile([P, 2, 512], F32, tag="stp")
            for j, s_ in enumerate(range(c0, c1)):
                kt = qb + s_
                masked = s_ in (0, 4)
                nc.tensor.matmul(
                    sT[:, j],
                    kT[off:off + 64, g // 2, kt * P:(kt + 1) * P],
                    qT[off:off + 64, jlo:jlo + 4, qb * P:(qb + 1) * P],
                    start=True, stop=not masked)
                if masked:
                    nc.tensor.matmul(
                        sT[:, j].rearrange("p (h a) -> p h a", h=4),
                        mT0[:] if s_ == 0 else mT4[:], identb4,
                        start=False, stop=True)
            nc.scalar.activation(
                e[:, c0:c1],
                sT[:, 0:n].rearrange("p j (h a) -> p j h a", h=4),
                ACTF.Exp, bias=0.0, scale=float(HD) ** -0.5)
        return e

    def emit_attnv(qb, g, e):
        off, jlo = pair_params(g)
        par = g % 2
        nT = ntp.tile([P, 512], F32, tag="ntp")
        for s_ in range(5):
            kt = qb + s_
            lhsT = vA[:, kt, g] if par == 0 else vB[:, kt, g]
            nc.tensor.matmul(
                nT[:], lhsT,
                e[:, s_].rearrange("p h a -> p (h a)"),
                start=(s_ == 0), stop=(s_ == 4))
        # numerator rows sit at partitions off:off+64, the denominator
        # replicated at the other 64. Reciprocal straight out of PSUM with
        # a partition-shifted write, then one fused normalize+evacuate.
        dof = 64 - off
        rsb = rpool.tile([P, 512], F32, tag="rsb")
        nc.vector.reciprocal(rsb[off:off + 64, :], nT[dof:dof + 64, :])
        nc.vector.tensor_tensor(
            out=oT[off:off + 64, jlo:jlo + 4, qb * P:(qb + 1) * P],
            in0=nT[off:off + 64, :].rearrange("p (h a) -> p h a", h=4),
            in1=rsb[off:off + 64, :].rearrange("p (h a) -> p h a", h=4),
            op=ALU.mult)

    y_dr = y_d.rearrange("(a p) n -> p a n", p=P)

    def emit_outproj_dh(at, dh):
        yps = pp.tile([P, 512], F32, tag="pp")
        for jt in range(8):
            nc.tensor.matmul(
                yps[:],
                oT[:, jt, at * P:(at + 1) * P],
                wo[:, jt, dh * 512:(dh + 1) * 512],
                start=(jt == 0), stop=(jt == 7))
        if dh == 0:
            nc.vector.tensor_copy(y_sb[:, at, 0:512], yps[:])
        else:
            nc.scalar.copy(y_sb[:, at, 512:1024], yps[:])
        nc.sync.dma_start(y_dr[:, at:at + 1, dh * 512:(dh + 1) * 512],
                          y_sb[:, at:at + 1, dh * 512:(dh + 1) * 512])

    def outproj_half_mm(at, yps0, yps1, half):
        for j in range(4):
            jt = half * 4 + j
            nc.tensor.matmul(yps0[:], oT[:, jt, at * P:(at + 1) * P],
                             wo[:, jt, 0:512],
                             start=(jt == 0), stop=(jt == 7))
            nc.tensor.matmul(yps1[:], oT[:, jt, at * P:(at + 1) * P],
                             wo[:, jt, 512:1024],
                             start=(jt == 0), stop=(jt == 7))

    # ---- interleaved schedule ----
    # kv tiles run upfront (cheap phase, Act mostly idle); during attention
    # the scores feed the Act exp queue while PE gaps are filled with the
    # next q projection and the previous block's out-projection.
    do_q = stage >= 1
    do_attn = stage >= 2 and stage != 6
    do_attnv = stage in (2, 3, 5, 6)
    do_out = stage == 3

    def sc(qb, g):
        if do_attn:
            return emit_scores(qb, g)
        return e_const

    if stage >= 1:
        # software pipeline: mm stages stream on the PE while each tile's
        # norm/rope chain (Act/DVE/Pool only) runs concurrently; all PE
        # transposes are emitted at the END of the preamble, by which time
        # their chains have completed - no in-order PE stalls. Tiles 5..7
        # norm/transpose lazily inside the attention loop (kT(5..7) are
        # first read at qb=1..3).
        emit_kv_mm(0)
        for ct in range(5):
            emit_kv_mm(ct + 1)
            emit_kv_norm(ct)
        q0a = q0b = None
        if do_q:
            # stage q0 in the scores pool (idle until the first score MM):
            # q0's matmuls then fill the PE gap while xt_q3 is still landing
            q0ps = stp.tile([P, 2, 512], F32, tag="stp")
            q0a = q0ps[:, 0]
            q0b = q0ps[:, 1]
            qproj_mm(0, q0a, q0b, dts=range(0, 8))
        emit_kv_mm(6)
        if do_q:
            qproj_tail(0, q0a, q0b)
        emit_kv_mm(7)
        for ct in range(5):
            emit_kv_transp(ct)
        if do_q:
            qproj_transpose(0, 0)
            qproj_transpose(0, 1)
        if stage < 2:
            for ct in range(5, 8):
                emit_kv_norm(ct)
                emit_kv_transp(ct)
    if stage >= 2:
        nc.sync.dma_start(wo[:], wo_d.rearrange("(a p) n -> p a n", p=P))
    if do_q and stage < 2:
        for at in range(1, NQT):
            emit_qproj(at)

    for qb in range(NQT if stage >= 2 else 0):
        last = qb == NQT - 1
        nxt = qb + 1
        qp = None
        if do_q and nxt < NQT:
            qpa = pp.tile([P, 512], F32, tag="pp")
            qpb = pp.tile([P, 512], F32, tag="pp")
            qp = (qpa, qpb)
        es = [None] * KV
        if qb == 0 and stage >= 2:
            emit_kv_norm(5)
        es[0] = sc(qb, 0)
        if qp:
            qproj_mm(nxt, *qp, dts=range(0, 4))
        es[1] = sc(qb, 1)
        if qb == 0 and stage >= 2:
            emit_kv_norm(6)
            emit_kv_transp(5)
        if qb == 1 and stage >= 2:
            emit_kv_norm(7)
            emit_kv_transp(6)
        if qb == 2 and stage >= 2:
            emit_kv_transp(7)
        if do_attnv:
            emit_attnv(qb, 0, e_const if stage in (5, 6) else es[0])
        if qp:
            qproj_mm(nxt, *qp, dts=range(4, 8))
            qproj_tail(nxt, *qp)
        if do_out and last:
            emit_outproj_dh(qb - 1, 0)
        es[2] = sc(qb, 2)
        if do_attnv:
            emit_attnv(qb, 1, e_const if stage in (5, 6) else es[1])
        if do_out and qb > 0 and not last:
            emit_outproj_dh(qb - 1, 0)
        if do_out and last:
            emit_outproj_dh(qb - 1, 1)
            # scores of the last group go out early so its exp latency
            # hides under the remaining PE work of this block
            es[3] = sc(qb, 3)
            # groups 0/1 normalized -> oT slots jt 0..3 complete; run the
            # first half of this block's own out-projection now so the
            # serial tail after the last attnv is just the second half.
            yl0 = pp.tile([P, 512], F32, tag="pp")
            yl1 = pp.tile([P, 512], F32, tag="pp")
            outproj_half_mm(qb, yl0, yl1, 0)
        if es[3] is None:
            es[3] = sc(qb, 3)
        if do_attnv:
            emit_attnv(qb, 2, e_const if stage in (5, 6) else es[2])
        if do_out and qb > 0 and not last:
            emit_outproj_dh(qb - 1, 1)
        if do_attnv:
            emit_attnv(qb, 3, e_const if stage in (5, 6) else es[3])
        if qp:
            qproj_transpose(nxt, 0)
            qproj_transpose(nxt, 1)
        if do_out and last:
            for j in range(4):
                jt = 4 + j
                nc.tensor.matmul(yl0[:], oT[:, jt, qb * P:(qb + 1) * P],
                                 wo[:, jt, 0:512],
                                 start=False, stop=(jt == 7))
            nc.vector.tensor_copy(y_sb[:, qb, 0:512], yl0[:])
            nc.sync.dma_start(y_dr[:, qb:qb + 1, 0:512],
                              y_sb[:, qb:qb + 1, 0:512])
            for j in range(4):
                jt = 4 + j
                nc.tensor.matmul(yl1[:], oT[:, jt, qb * P:(qb + 1) * P],
                                 wo[:, jt, 512:1024],
                                 start=False, stop=(jt == 7))
            nc.scalar.copy(y_sb[:, qb, 512:1024], yl1[:])
            nc.sync.dma_start(y_dr[:, qb:qb + 1, 512:1024],
                              y_sb[:, qb:qb + 1, 512:1024])

    if stage == 3:
        pass  # y streamed per query block
    elif stage != 3:
        # timing-only variants: keep the output transfer but source bytes
        # from xt (y_sb is never written below stage 3)
        nc.sync.dma_start(y_d.rearrange("(a p) n -> p a n", p=P),
                          xt[:].bitcast(F32))


def build_program(loop_n=None, stage=3):
    """loop_n: if given, wrap the whole kernel body in a hardware For_i loop
    executing it loop_n times back-to-back (used by test.py to measure true
    per-execution device time via the two-point slope method, which cancels
    the fixed multi-ms dispatch/tunnel latency of this environment)."""
    nc = bass.Bass()
    with tile.TileContext(nc) as tc:
        with ExitStack() as ctx:
            if loop_n is None:
                emit(nc, tc, ctx, stage=stage)
            else:
                with tc.For_i(0, loop_n, 1, hint_engines=(
                        mybir.EngineType.PE, mybir.EngineType.Activation,
                        mybir.EngineType.DVE, mybir.EngineType.Pool,
                        mybir.EngineType.SP)):
                    emit(nc, tc, ctx, stage=stage)
    split_multiwaits(nc)
    return nc


_NC = None


def _get_program():
    global _NC
    if _NC is None:
        _NC = build_program()
    return _NC


# ---------------- host-side prep ----------------

def prep_core_inputs(x, wq, wk, wv, wo, qn_w, kn_w):
    bf = ml_dtypes.bfloat16
    perm = np.concatenate([np.arange(0, 64, 2), np.arange(1, 64, 2)])

    wq_p = np.ascontiguousarray(
        wq.reshape(D, H, HD)[:, HEAD_OF_SLOT][:, :, perm].reshape(D, H * HD)
    ).astype(bf)
    wk_p = wk.reshape(D, KV, HD)[:, :, perm].reshape(D, KV * HD)
    wkv_p = np.ascontiguousarray(
        np.concatenate([wk_p, wv], axis=1)).astype(bf)
    wo_p = np.ascontiguousarray(
        wo.reshape(H, HD, D)[HEAD_OF_SLOT].reshape(H * HD, D)).astype(bf)

    inv_freq = 1.0 / (10000.0 ** (np.arange(0, HD, 2, dtype=np.float64) / HD))
    freq64 = np.concatenate([inv_freq, inv_freq])  # emb[t, d] = t * freq64[d]

    def rope_tables(tvec, w):
        ang = tvec[:, None].astype(np.float64) * freq64[None, :]
        c = np.cos(ang).astype(np.float32)
        s_ = np.sin(ang).astype(np.float32)
        we, wo_ = w[0::2], w[1::2]
        cosT = np.concatenate([we[None] * c[:, 0::2], wo_[None] * c[:, 1::2]], axis=1)
        sinT = np.concatenate([wo_[None] * s_[:, 0::2], we[None] * s_[:, 1::2]], axis=1)
        return cosT, sinT

    in_maps = []
    for core in range(NCORES):
        b, ci = divmod(core, NCORES // B)
        q_lo = ci * TQ
        c_lo = q_lo - WINDOW
        ctx_blk = np.zeros((TC, D), np.float32)
        lo = max(c_lo, 0)
        ctx_blk[lo - c_lo:, :] = x[b, lo:q_lo + TQ, :]
        xt_c = np.ascontiguousarray(
            ctx_blk.T.reshape(8, P, 4, 256).transpose(2, 1, 0, 3)).astype(bf)

        tq = np.arange(q_lo, q_lo + TQ)
        cq, sq = rope_tables(tq, qn_w)
        cosq_c = cq.reshape(NQT, P, HD).transpose(1, 0, 2)
        sinq_c = sq.reshape(NQT, P, HD).transpose(1, 0, 2)
        tk = np.arange(c_lo, c_lo + TC)
        ck, sk = rope_tables(tk, kn_w)
        cosk_c = ck.reshape(NCT, P, HD).transpose(1, 0, 2)
        sink_c = sk.reshape(NCT, P, HD).transpose(1, 0, 2)
        rope_c = np.ascontiguousarray(np.concatenate(
            [cosq_c, sinq_c, cosk_c, sink_c], axis=1)).astype(bf)

        vm = np.ones((P, NCT), np.float32)
        n_pad_tiles = (lo - c_lo) // P
        vm[:, :n_pad_tiles] = 0.0
        vden_c = np.ascontiguousarray(
            np.repeat(vm[:, :, None], 64, axis=2)).astype(bf)

        in_maps.append({
            "xt": xt_c, "wq": wq_p, "wkv": wkv_p, "wo": wo_p,
            "rope": rope_c, "vden": vden_c,
        })
    return in_maps


def kernel(x, wq, wk, wv, wo, qn_w, kn_w):
    from concourse.bass_utils import run_bass_kernel_spmd
    in_maps = prep_core_inputs(x, wq, wk, wv, wo, qn_w, kn_w)
    nc = _get_program()
    res = run_bass_kernel_spmd(nc, in_maps, list(range(NCORES)))
    out = np.empty((B, S, D), np.float32)
    for core in range(NCORES):
        b, ci = divmod(core, NCORES // B)
        out[b, ci * TQ:(ci + 1) * TQ, :] = res.results[core]["y"]
    return out
